# revision 1
# baseline (speedup 1.0000x reference)
"""Data-parallel GCN classifier kernel for 8 trn2 NeuronCores.

Strategy (per sharding hint): pure data parallel — shard batch B=4096 across
8 cores (512/core), params replicated. The edge gather/scatter is folded on
host into a dense 64x64 normalized adjacency matrix A_hat (A+I with symmetric
deg^-1/2 normalization), so on-device the GNN is two small dense matmul chains.
BatchNorm (training-mode, stats over (B, C) per node) is computed with GLOBAL
stats: the model is jit-compiled under GSPMD with batch-sharded inputs, so XLA
inserts the cross-core all-reduces for the BN means exactly.

Tiers (first that works wins):
  A) jax.jit + NamedSharding over 8 device batch shards (exact BN, 8 cores)
  B) single-device jax.jit (exact, 1 core)
  C) numpy on host (exact, fallback of last resort)
"""

import sys

import numpy as np

EPS = 1e-5
B, N, FIN, D_FP, OUT = 4096, 64, 67, 2048, 2
N_CORES = 8


def _build_ahat(edge_list: np.ndarray) -> np.ndarray:
    """Dense normalized adjacency (A + I with GCN deg^-1/2 norm), [dst, src]."""
    el = np.asarray(edge_list)
    loops = np.arange(N, dtype=el.dtype)
    src = np.concatenate([el[0], loops]).astype(np.int64)
    dst = np.concatenate([el[1], loops]).astype(np.int64)
    deg = np.zeros((N,), np.float64)
    np.add.at(deg, dst, 1.0)
    dinv = np.where(deg > 0, 1.0 / np.sqrt(deg), 0.0)
    a = np.zeros((N, N), np.float64)
    np.add.at(a, (dst, src), dinv[src] * dinv[dst])
    return a.astype(np.float32)


def _model_np(x_fingerprints, x_node_features, ahat, W1, b1, g1, be1,
              W2, b2, g2, be2, Wl1, bl1, Wl2, bl2, Wfc, bfc):
    x = np.asarray(x_node_features, np.float32)
    t1 = np.einsum('bnf,of->bno', x, W1, optimize=True)
    g = np.einsum('ds,bso->bdo', ahat, t1, optimize=True) + b1
    m = g.mean(axis=(0, 2), keepdims=True)
    v = np.square(g - m).mean(axis=(0, 2), keepdims=True)
    g = (g - m) / np.sqrt(v + EPS) * g1[None, :, None] + be1[None, :, None]
    g = np.maximum(g, 0)
    t2 = np.einsum('bno,po->bnp', g, W2, optimize=True)
    g = np.einsum('ds,bsp->bdp', ahat, t2, optimize=True) + b2
    m = g.mean(axis=(0, 2), keepdims=True)
    v = np.square(g - m).mean(axis=(0, 2), keepdims=True)
    g = (g - m) / np.sqrt(v + EPS) * g2[None, :, None] + be2[None, :, None]
    g = np.maximum(g, 0)
    pooled = g.max(axis=1)
    h = np.maximum(x_fingerprints @ Wl1.T + bl1, 0)
    h = np.maximum(h @ Wl2.T + bl2, 0)
    return (np.concatenate([pooled, h], axis=1) @ Wfc.T + bfc).astype(np.float32)


def _run_jax(inputs: dict, ahat: np.ndarray, n_devices: int) -> np.ndarray:
    import jax
    import jax.numpy as jnp

    def model(x_fp, x, ah, W1, b1, g1, be1, W2, b2, g2, be2,
              Wl1, bl1, Wl2, bl2, Wfc, bfc):
        t1 = jnp.einsum('bnf,of->bno', x, W1)
        g = jnp.einsum('ds,bso->bdo', ah, t1) + b1
        m = jnp.mean(g, axis=(0, 2), keepdims=True)
        v = jnp.mean(jnp.square(g - m), axis=(0, 2), keepdims=True)
        g = (g - m) * jax.lax.rsqrt(v + EPS) * g1[None, :, None] + be1[None, :, None]
        g = jax.nn.relu(g)
        t2 = jnp.einsum('bno,po->bnp', g, W2)
        g = jnp.einsum('ds,bsp->bdp', ah, t2) + b2
        m = jnp.mean(g, axis=(0, 2), keepdims=True)
        v = jnp.mean(jnp.square(g - m), axis=(0, 2), keepdims=True)
        g = (g - m) * jax.lax.rsqrt(v + EPS) * g2[None, :, None] + be2[None, :, None]
        g = jax.nn.relu(g)
        pooled = jnp.max(g, axis=1)
        h = jax.nn.relu(x_fp @ Wl1.T + bl1)
        h = jax.nn.relu(h @ Wl2.T + bl2)
        return jnp.concatenate([pooled, h], axis=1) @ Wfc.T + bfc

    params = [np.asarray(inputs[k], np.float32) for k in
              ('W1', 'b1', 'g1', 'be1', 'W2', 'b2', 'g2', 'be2',
               'Wl1', 'bl1', 'Wl2', 'bl2', 'Wfc', 'bfc')]
    x_fp = np.asarray(inputs['x_fingerprints'], np.float32)
    x_nf = np.asarray(inputs['x_node_features'], np.float32)

    if n_devices > 1:
        from jax.sharding import Mesh, NamedSharding, PartitionSpec as P
        devices = jax.devices()[:n_devices]
        mesh = Mesh(np.asarray(devices), ('b',))
        shard_b = NamedSharding(mesh, P('b'))
        repl = NamedSharding(mesh, P())
        x_fp_d = jax.device_put(x_fp, shard_b)
        x_nf_d = jax.device_put(x_nf, shard_b)
        ah_d = jax.device_put(ahat, repl)
        params_d = [jax.device_put(p, repl) for p in params]
        fn = jax.jit(model, out_shardings=shard_b)
        out = fn(x_fp_d, x_nf_d, ah_d, *params_d)
    else:
        fn = jax.jit(model)
        out = fn(x_fp, x_nf, ahat, *params)
    out = np.asarray(jax.block_until_ready(out), np.float32)
    if not np.all(np.isfinite(out)):
        raise RuntimeError("non-finite output from jax path")
    return out


def kernel(**inputs) -> np.ndarray:
    ahat = _build_ahat(inputs['edge_list'])
    # Tier A: 8-core data parallel under GSPMD (exact global BN via all-reduce).
    try:
        import jax
        if len(jax.devices()) >= N_CORES:
            return _run_jax(inputs, ahat, N_CORES)
    except Exception as e:  # noqa: BLE001
        print(f"kernel: 8-core jax path failed ({type(e).__name__}: {e}); "
              f"falling back", file=sys.stderr)
    # Tier B: single device.
    try:
        return _run_jax(inputs, ahat, 1)
    except Exception as e:  # noqa: BLE001
        print(f"kernel: single-core jax path failed ({type(e).__name__}: {e}); "
              f"falling back to numpy", file=sys.stderr)
    # Tier C: exact numpy.
    p = {k: np.asarray(inputs[k], np.float32) for k in inputs if k != 'edge_list'}
    return _model_np(p['x_fingerprints'], p['x_node_features'], ahat,
                     p['W1'], p['b1'], p['g1'], p['be1'],
                     p['W2'], p['b2'], p['g2'], p['be2'],
                     p['Wl1'], p['bl1'], p['Wl2'], p['bl2'],
                     p['Wfc'], p['bfc'])


if __name__ == '__main__':
    rng = np.random.default_rng(0)
    demo = {
        'x_fingerprints': rng.standard_normal((B, D_FP), dtype=np.float32),
        'x_node_features': rng.standard_normal((B, N, FIN), dtype=np.float32),
        'edge_list': rng.integers(0, N, size=(2, 512)).astype(np.int32),
    }
    for name, shape, scale in [
        ('W1', (64, FIN), 0.1), ('b1', (64,), 0.1), ('g1', (N,), 0.1),
        ('be1', (N,), 0.1), ('W2', (32, 64), 0.1), ('b2', (32,), 0.1),
        ('g2', (N,), 0.1), ('be2', (N,), 0.1), ('Wl1', (400, D_FP), 0.025),
        ('bl1', (400,), 0.1), ('Wl2', (64, 400), 0.1), ('bl2', (64,), 0.1),
        ('Wfc', (OUT, 96), 0.1), ('bfc', (OUT,), 0.1),
    ]:
        demo[name] = (rng.standard_normal(shape) * scale).astype(np.float32)
    out = kernel(**demo)
    print('demo output', out.shape, out.dtype, float(np.abs(out).max()))
